# revision 1
# baseline (speedup 1.0000x reference)
"""Trainium2 Bass kernel for the news-attention module.

Computes, per batch b:
    hist = [history_repr | pos_emb[positions]]            [H, 500]
    cand = [candidate_repr | pos_emb[1]]                  [N, 500]
    hc = cand @ Wc.T ; hh = hist @ Wh.T                   [*, 200]
    a[n,h] = w2 . relu(hc[n] + hh[h] + b1)
    alpha = softmax_h(mask ? a : -1e9)
    out1 = alpha @ hist ; out2 = cand

Structure:
  - position gather folded into matmuls: pos part of hh = onehot(pos) @ E
    with E = pos_emb @ Wh2.T; candidate pos part + b1 folded into a
    per-partition bias column c0 applied during PSUM evacuation.
  - all fp32 matmuls stream as float32r (full-rate fp32 mode for moving
    free dim >= 256); psum accumulation is fp32 either way.
  - hidden built in [A-chunk, (n, h)] layout with zero-stride broadcast
    APs on the DVE; hc stored with each element duplicated (pairs) so the
    innermost AP dim is packed, enabling the DVE 2-byte fast path in bf16
    mode; relu in place.
  - w2 contraction in column form: lhsT = hidden chunk (100 pairs),
    rhs = w2 column, accumulating into a [100, 25] psum per batch ->
    logits leave PSUM via one cheap 100-lane copy instead of 40
    single-lane row copies.
  - softmax over h batched for all batches in [25, parity, b, h] layout
    (n = 2c + parity); mask bias tile built in the same layout.
  - alpha @ hist as two parity matmuls per batch with 1/sum folded into
    the PSUM-evacuation scale.

Sharding: data-parallel over batch, 8 batches per core on 8 cores.
Params replicated. Full inputs in, full outputs out.
"""

import sys

for _p in ("/opt/trn_rl_repo",):
    if _p not in sys.path:
        sys.path.insert(0, _p)

import numpy as np

import concourse.bass as bass
import concourse.bacc as bacc
import concourse.tile as tile
from concourse import mybir
from concourse import bass_utils
from concourse.masks import make_identity

DT = mybir.dt.float32
FR = mybir.dt.float32r
BF = mybir.dt.bfloat16
I32 = mybir.dt.int32
AF = mybir.ActivationFunctionType
ALU = mybir.AluOpType
AX = mybir.AxisListType

NCORES = 8
B = 64
BC = B // NCORES  # 8 batches per core
H = 50
N = 50
D = 400
P = 100
A = 200
F = D + P       # 500
J = 52
NC2 = N // 2    # 25 pair-chunks of 100 pairs (2 candidates) per batch

# bf16 hidden pipeline (hc/hh/hidden/w2 in bf16; everything else fp32)
USE_BF16 = False
SKIP = set()  # timing ablations: {"hidden","matvec","gemm","transp","final"}
MATVEC_ROW = False  # row-form matvec (fewer, wider PE ops + ACT evac)
HDT = BF if USE_BF16 else DT


def _bc(v, pos, n):
    """Insert a zero-stride (broadcast) dim of length n at position pos."""
    ap = [list(x) for x in v.ap]
    ap.insert(pos, [0, n])
    return bass.AP(tensor=v.tensor, offset=v.offset, ap=ap)


def _ap(v, offset_delta, ap_list):
    return bass.AP(tensor=v.tensor, offset=v.offset + offset_delta, ap=ap_list)


def _r(ap):
    """Placeholder for fp32->float32r streaming (needs producer-side
    rounding on HW; disabled)."""
    return ap


def _body(nc, hist_in, cand_in, mask_in, pos_in, pos_emb, w1t, pos_embT,
          b1, w2, ur_out, cand_out, tc):
    import contextlib

    ctx = contextlib.ExitStack()
    with ctx:
        consts = ctx.enter_context(tc.tile_pool(name="consts", bufs=1))
        ps = ctx.enter_context(tc.tile_pool(name="ps", bufs=4, space="PSUM"))
        psm = ctx.enter_context(tc.tile_pool(name="psm", bufs=2, space="PSUM"))
        hidp = ctx.enter_context(tc.tile_pool(name="hid", bufs=3))
        smp = ctx.enter_context(tc.tile_pool(name="smp", bufs=1))
        amcp = ctx.enter_context(tc.tile_pool(name="amcp", bufs=2))
        eTp = ctx.enter_context(tc.tile_pool(name="eTp", bufs=3))

        # ---------------- constants ----------------
        ident = consts.tile([128, 128], DT)
        make_identity(nc, ident)

        # W1T[f, a] in 10 f-chunks of 100 (host provides W1 transposed)
        w1T = consts.tile([100, 10, A], DT)
        nc.sync.dma_start(out=w1T,
                          in_=w1t.ap().rearrange("(k p) a -> p k a", p=100))

        pos_emb_s = consts.tile([J, P], DT)
        nc.sync.dma_start(out=pos_emb_s, in_=pos_emb.ap())
        posT = consts.tile([P, J], DT)
        nc.sync.dma_start(out=posT, in_=pos_embT.ap())

        # E[j, a] = pos_emb @ Wh2.T  (Wh2 = W1[:, 900:1000])
        E_s = consts.tile([J, A], DT)
        psE = ps.tile([J, A], DT, tag="ps")
        nc.tensor.matmul(psE, lhsT=_r(posT[:, :]), rhs=_r(w1T[:, 9, :]),
                         start=True, stop=True)
        nc.vector.tensor_copy(out=E_s, in_=psE)

        b1row = consts.tile([1, A], DT)
        nc.sync.dma_start(out=b1row, in_=b1.ap())
        one11 = consts.tile([1, 1], DT)
        nc.vector.memset(one11, 1.0)

        # c0[a] = Wc2 @ pos_emb[1] + b1 as two per-partition bias columns
        c0col = consts.tile([100, 2], DT)
        for ac in range(2):
            asl = slice(ac * 100, (ac + 1) * 100)
            psc = ps.tile([100, 1], DT, tag="ps")
            nc.tensor.matmul(psc, lhsT=_r(w1T[:, 4, asl]), rhs=_r(posT[:, 1:2]),
                             start=True, stop=False)
            nc.tensor.matmul(psc, lhsT=_r(b1row[:, asl]), rhs=_r(one11[:, :]),
                             start=False, stop=True)
            nc.scalar.copy(out=c0col[:, ac:ac + 1], in_=psc)

        w2col = consts.tile([100, 2], HDT)
        nc.gpsimd.dma_start(out=w2col,
                            in_=w2.ap().rearrange("(c p) -> p c", p=100))

        # mask bias (mask-1)*1e9 in [c, q, b, h] layout (broadcast over c, q)
        mb25 = consts.tile([NC2, 2, BC, H], DT)
        nc.gpsimd.dma_start(out=mb25, in_=_bc(_bc(mask_in.ap(), 0, 2), 0, NC2))
        nc.scalar.activation(out=mb25, in_=mb25, func=AF.Copy,
                             bias=-1e9, scale=1e9)

        # one-hot of positions, transposed: onehot[j, b*H+h] = (pos[b,h]==j)
        pos52 = consts.tile([J, BC * H], I32)
        nc.gpsimd.dma_start(out=pos52, in_=_bc(pos_in.ap(), 0, J))
        iot = consts.tile([J, BC * H], I32)
        nc.gpsimd.iota(iot, pattern=[[0, BC * H]], base=0, channel_multiplier=1)
        onehot_s = consts.tile([J, BC * H], DT)
        nc.vector.tensor_tensor(out=onehot_s, in0=iot, in1=pos52, op=ALU.is_equal)

        # ---------------- data load + transpose ----------------
        cand_all = consts.tile([100, 4, D], DT)   # [2x50 rows, batch-pair, feat]
        hist_all = consts.tile([100, 4, D], DT)
        for hf in range(2):
            sl = slice(hf * 50, (hf + 1) * 50)
            src_c = _ap(cand_in.ap(), hf * N * D,
                        [[D, 50], [2 * N * D, 4], [1, D]])
            src_h = _ap(hist_in.ap(), hf * H * D,
                        [[D, 50], [2 * H * D, 4], [1, D]])
            nc.sync.dma_start(out=cand_all[sl, :, :], in_=src_c)
            nc.sync.dma_start(out=hist_all[sl, :, :], in_=src_h)

        candT = consts.tile([100, 4, BC * N], DT)  # [feat-chunk, k, (b,n)]
        histT = consts.tile([100, 4, BC * H], DT)
        for g in range(4 if "transp" not in SKIP else 0):
            ptc = ps.tile([100, 4, 100], DT, tag="ps")
            pth = ps.tile([100, 4, 100], DT, tag="ps")
            for k in range(4):
                nc.tensor.transpose(
                    ptc[:, k, :],
                    _r(cand_all[:, g, k * 100:(k + 1) * 100]),
                    _r(ident[:100, :100]))
                nc.tensor.transpose(
                    pth[:, k, :],
                    _r(hist_all[:, g, k * 100:(k + 1) * 100]),
                    _r(ident[:100, :100]))
            nc.scalar.copy(out=candT[:, :, g * 100:(g + 1) * 100], in_=ptc)
            nc.scalar.copy(out=histT[:, :, g * 100:(g + 1) * 100], in_=pth)

        # hist with position columns, natural layout, all batches
        histf_all = consts.tile([H, BC, F], DT)
        src_hf = _ap(hist_in.ap(), 0, [[D, H], [H * D, BC], [1, D]])
        nc.sync.dma_start(out=histf_all[:, :, 0:D], in_=src_hf)
        for b in range(BC):
            ppg = ps.tile([H, P], DT, tag="ps")
            nc.tensor.matmul(ppg, lhsT=_r(onehot_s[:, b * H:(b + 1) * H]),
                             rhs=_r(pos_emb_s[:, :]), start=True, stop=True)
            nc.scalar.copy(out=histf_all[:, b, D:F], in_=ppg)

        # candidate passthrough: two strided DMAs + pos_emb[1] broadcast
        for hf in range(2):
            dst = _ap(cand_out.ap(), hf * N * F,
                      [[F, 50], [2 * N * F, 4], [1, D]])
            nc.sync.dma_start(out=dst, in_=cand_all[hf * 50:(hf + 1) * 50, :, :])
        nc.gpsimd.dma_start(
            out=cand_out.ap()[:, :, D:F],
            in_=_bc(_bc(pos_emb.ap()[1:2, :], 0, N), 0, BC))

        # ------- GEMMs: hcT2[a, (b,n), dup2] (duplicated), hhT[a, (b,h)] ----
        hcT2 = consts.tile([100, 2, BC * N, 2], HDT)
        hhT = consts.tile([100, 2, BC * H], HDT)
        for ac in range(2 if "gemm" not in SKIP else 0):
            asl = slice(ac * 100, (ac + 1) * 100)
            pg = ps.tile([100, BC * N], DT, tag="ps")
            for k in range(4):
                nc.tensor.matmul(pg, lhsT=_r(w1T[:, k, asl]),
                                 rhs=_r(candT[:, k, :]),
                                 start=(k == 0), stop=(k == 3))
            # evacuate + add c0 bias, duplicating each element (dup2 dim)
            nc.scalar.activation(out=hcT2[:, ac, :, :], in_=_bc(pg[:, :], 2, 2),
                                 func=AF.Identity, bias=c0col[:, ac:ac + 1],
                                 scale=1.0)

            ph = ps.tile([100, BC * H], DT, tag="ps")
            for k in range(4):
                nc.tensor.matmul(ph, lhsT=_r(w1T[:, 5 + k, asl]),
                                 rhs=_r(histT[:, k, :]),
                                 start=(k == 0), stop=False)
            nc.tensor.matmul(ph, lhsT=_r(E_s[:, asl]), rhs=_r(onehot_s[:, :]),
                             start=False, stop=True)
            nc.scalar.copy(out=hhT[:, ac, :], in_=ph)

        # ---------------- hidden + relu + w2 column matvec ----------------
        scratch = None
        if MATVEC_ROW:
            scratch = nc.dram_tensor(
                f"scratch{nc.next_id()}", [BC, N, H], DT)
        amr = smp.tile([NC2, 2, BC, H], DT, tag="amr")
        for b in range(BC):
            hids = []
            for ac in range(2):
                # hc broadcast over h via duplicated pairs: free = (n, hq, hr)
                v = hcT2[:, ac, b * N:(b + 1) * N, :]   # [[p],[2,50],[1,2]]
                hcb = _bc(v, 2, H // 2)                 # [100, 50, 25, 2]
                w = hhT[:, ac, b * H:(b + 1) * H]       # [[p],[1,50]]
                hhb = _ap(w, 0, [list(w.ap[0]), [0, N], [2, H // 2], [1, 2]])
                hid = hidp.tile([100, N, H // 2, 2], HDT, tag=f"hid{ac}")
                if "hidden" not in SKIP:
                    nc.vector.tensor_add(out=hid, in0=hcb, in1=hhb)
                    nc.vector.tensor_scalar_max(out=hid, in0=hid, scalar1=0.0)
                hids.append(hid)
            if MATVEC_ROW:
                arow = amcp.tile([1, 5, 500], DT, tag="arow")
                for gn in range(5 if "matvec" not in SKIP else 0):
                    psa = psm.tile([1, 500], DT, tag="psa")
                    for ac in range(2):
                        hv = hids[ac]
                        rhs = hv[:, gn * 10:(gn + 1) * 10, :, :]
                        nc.tensor.matmul(psa, lhsT=_r(w2col[:, ac:ac + 1]),
                                         rhs=_r(rhs),
                                         start=(ac == 0), stop=(ac == 1))
                    nc.scalar.copy(out=arow[0:1, gn, :], in_=psa)
                if "matvec" in SKIP:
                    continue
                nc.sync.dma_start(out=scratch.ap()[b], in_=arow)
                continue
            amc = psm.tile([100, NC2], DT, tag="amc")
            for c in range(NC2 if "matvec" not in SKIP else 0):
                for ac in range(2):
                    hv = hids[ac]
                    pst = [list(x) for x in hv.ap][0]
                    lhs = _ap(hv, c * 100, [pst, [1, 100]])
                    nc.tensor.matmul(amc[:, c:c + 1], lhsT=_r(lhs),
                                     rhs=_r(w2col[:, ac:ac + 1]),
                                     start=(ac == 0), stop=(ac == 1))
            amcs = amcp.tile([100, NC2], DT, tag="amcs")
            if "matvec" in SKIP:
                continue
            nc.vector.tensor_copy(out=amcs, in_=amc)
            amT = psm.tile([NC2, 2 * H], DT, tag="amT")
            nc.tensor.transpose(amT[:, :], _r(amcs[:, :]),
                                _r(ident[:100, :100]))
            nc.vector.tensor_copy(out=amr[:, :, b, :], in_=amT)
        if MATVEC_ROW and "matvec" not in SKIP:
            for q in range(2):
                src_q = _ap(scratch.ap(), q * H,
                            [[2 * H, NC2], [N * H, BC], [1, H]])
                nc.sync.dma_start(out=amr[:, q, :, :], in_=src_q)

        # ---------------- batched mask + softmax over h ----------------
        amm = smp.tile([NC2, 2, BC, H], DT, tag="amm")
        nc.vector.tensor_add(out=amm, in0=amr, in1=mb25)
        nm = smp.tile([NC2, 2, BC], DT, tag="nm")
        nc.vector.tensor_reduce(out=nm, in_=amm, axis=AX.X, op=ALU.max,
                                negate=True)
        am2 = smp.tile([NC2, 2, BC, H], DT, tag="am2")
        nc.vector.tensor_add(out=am2, in0=amm, in1=_bc(nm[:, :, :], 3, H))
        ex = smp.tile([NC2, 2, BC, H], DT, tag="ex")
        nc.scalar.activation(out=ex, in_=am2, func=AF.Exp)
        ssum = smp.tile([NC2, 2, BC], DT, tag="ssum")
        nc.vector.tensor_reduce(out=ssum, in_=ex, axis=AX.X, op=ALU.add)
        rs = smp.tile([NC2, 2, BC], DT, tag="rs")
        nc.vector.reciprocal(rs, ssum)

        # ---------------- attention-weighted history ----------------
        urs_all = consts.tile([NC2, 2, BC, F], DT)
        for b in range(BC if "final" not in SKIP else 0):
            for q in range(2):
                peT = ps.tile([H, NC2], DT, tag="ps")
                nc.tensor.transpose(peT[:, :], _r(ex[:, q, b, :]),
                                    _r(ident[:NC2, :NC2]))
                eT = eTp.tile([H, NC2], DT, tag="eT")
                nc.vector.tensor_copy(out=eT, in_=peT)

                pur = ps.tile([NC2, F], DT, tag="ps")
                nc.tensor.matmul(pur, lhsT=_r(eT[:, :]),
                                 rhs=_r(histf_all[:, b, :]),
                                 start=True, stop=True)
                nc.scalar.activation(out=urs_all[:, q, b, :], in_=pur,
                                     func=AF.Copy, scale=rs[:, q, b:b + 1])
        dst_ur = _ap(ur_out.ap(), 0,
                     [[2 * F, NC2], [F, 2], [N * F, BC], [1, F]])
        nc.sync.dma_start(out=dst_ur, in_=urs_all)


def build(debug=False, reps=1):
    nc = bacc.Bacc("TRN2", target_bir_lowering=False, debug=debug)
    hist_in = nc.dram_tensor("hist_in", [BC, H, D], DT, kind="ExternalInput")
    cand_in = nc.dram_tensor("cand_in", [BC, N, D], DT, kind="ExternalInput")
    mask_in = nc.dram_tensor("mask_in", [BC, H], DT, kind="ExternalInput")
    pos_in = nc.dram_tensor("pos_in", [BC, H], I32, kind="ExternalInput")
    pos_emb = nc.dram_tensor("pos_emb", [J, P], DT, kind="ExternalInput")
    w1t = nc.dram_tensor("w1t", [2 * F, A], DT, kind="ExternalInput")
    pos_embT = nc.dram_tensor("pos_embT", [P, J], DT, kind="ExternalInput")
    b1 = nc.dram_tensor("b1", [A], DT, kind="ExternalInput")
    w2 = nc.dram_tensor("w2", [A], DT, kind="ExternalInput")
    ur_out = nc.dram_tensor("ur_out", [BC, N, F], DT, kind="ExternalOutput")
    cand_out = nc.dram_tensor("cand_out", [BC, N, F], DT, kind="ExternalOutput")

    with tile.TileContext(nc) as tc:
        for _ in range(reps):
            _body(nc, hist_in, cand_in, mask_in, pos_in, pos_emb, w1t,
                  pos_embT, b1, w2, ur_out, cand_out, tc)
    nc.compile()
    return nc


_NC = None


def _get_nc():
    global _NC
    if _NC is None:
        _NC = build(debug=False)
    return _NC


def make_in_maps(history_repr, candidate_repr, user_history_mask,
                 user_history_position, pos_emb, W1, b1, w2):
    hist = np.ascontiguousarray(np.asarray(history_repr, np.float32))
    cand = np.ascontiguousarray(np.asarray(candidate_repr, np.float32))
    mask = np.asarray(user_history_mask).astype(np.float32)
    pos = np.asarray(user_history_position).astype(np.int32)
    pe = np.ascontiguousarray(np.asarray(pos_emb, np.float32))
    w1t = np.ascontiguousarray(np.asarray(W1, np.float32).T)
    peT = np.ascontiguousarray(pe.T)
    b1_ = np.ascontiguousarray(np.asarray(b1, np.float32))
    w2_ = np.ascontiguousarray(np.asarray(w2, np.float32))
    in_maps = []
    for c in range(NCORES):
        sl = slice(c * BC, (c + 1) * BC)
        in_maps.append({
            "hist_in": hist[sl], "cand_in": cand[sl],
            "mask_in": mask[sl], "pos_in": pos[sl],
            "pos_emb": pe, "w1t": w1t, "pos_embT": peT,
            "b1": b1_, "w2": w2_,
        })
    return in_maps


def kernel(history_repr, candidate_repr, user_history_mask,
           user_history_position, pos_emb, W1, b1, w2, b2=None, **_ignored):
    # b2 shifts every logit equally -> cancels in softmax; unused.
    nc = _get_nc()
    in_maps = make_in_maps(history_repr, candidate_repr, user_history_mask,
                           user_history_position, pos_emb, W1, b1, w2)
    res = bass_utils.run_bass_kernel_spmd(nc, in_maps, list(range(NCORES)))
    ur = np.concatenate([res.results[c]["ur_out"] for c in range(NCORES)], 0)
    cand = np.concatenate([res.results[c]["cand_out"] for c in range(NCORES)], 0)
    return ur, cand



# revision 51
# speedup vs baseline: 2.3071x; 2.3071x over previous
"""Trainium2 Bass kernel for the news-attention module.

Computes, per batch b:
    hist = [history_repr | pos_emb[positions]]            [H, 500]
    cand = [candidate_repr | pos_emb[1]]                  [N, 500]
    hc = cand @ Wc.T ; hh = hist @ Wh.T                   [*, 200]
    a[n,h] = w2 . relu(hc[n] + hh[h] + b1)
    alpha = softmax_h(mask ? a : -1e9)
    out1 = alpha @ hist ; out2 = cand

Structure:
  - position gather folded into matmuls: pos part of hh = onehot(pos) @ E
    with E = pos_emb @ Wh2.T; candidate pos part + b1 folded into a
    per-partition bias column c0 applied during PSUM evacuation.
  - all fp32 matmuls stream as float32r (full-rate fp32 mode for moving
    free dim >= 256); psum accumulation is fp32 either way.
  - hidden built in [A-chunk, (n, h)] layout with zero-stride broadcast
    APs on the DVE; hc stored with each element duplicated (pairs) so the
    innermost AP dim is packed, enabling the DVE 2-byte fast path in bf16
    mode; relu in place.
  - w2 contraction in column form: lhsT = hidden chunk (100 pairs),
    rhs = w2 column, accumulating into a [100, 25] psum per batch ->
    logits leave PSUM via one cheap 100-lane copy instead of 40
    single-lane row copies.
  - softmax over h batched for all batches in [25, parity, b, h] layout
    (n = 2c + parity); mask bias tile built in the same layout.
  - alpha @ hist as two parity matmuls per batch with 1/sum folded into
    the PSUM-evacuation scale.

Sharding: data-parallel over batch, 8 batches per core on 8 cores.
Params replicated. Full inputs in, full outputs out.
"""

import sys

for _p in ("/opt/trn_rl_repo",):
    if _p not in sys.path:
        sys.path.insert(0, _p)

import numpy as np

import concourse.bass as bass
import concourse.bacc as bacc
import concourse.tile as tile
from concourse import mybir
from concourse import bass_utils
from concourse.masks import make_identity

DT = mybir.dt.float32
FR = mybir.dt.float32r
BF = mybir.dt.bfloat16
I32 = mybir.dt.int32
AF = mybir.ActivationFunctionType
ALU = mybir.AluOpType
AX = mybir.AxisListType

NCORES = 8
B = 64
BC = B // NCORES  # 8 batches per core
H = 50
N = 50
D = 400
P = 100
A = 200
F = D + P       # 500
J = 52
NC2 = N // 2    # 25 pair-chunks of 100 pairs (2 candidates) per batch

# bf16 hidden pipeline (hc/hh/hidden/w2 in bf16; everything else fp32)
USE_BF16 = True
SKIP = set()  # timing ablations: {"hidden","matvec","gemm","transp","final"}
MATVEC_ROW = False  # row-form matvec (fewer, wider PE ops + ACT evac)
HDT = BF if USE_BF16 else DT


def _bc(v, pos, n):
    """Insert a zero-stride (broadcast) dim of length n at position pos."""
    ap = [list(x) for x in v.ap]
    ap.insert(pos, [0, n])
    return bass.AP(tensor=v.tensor, offset=v.offset, ap=ap)


def _ap(v, offset_delta, ap_list):
    return bass.AP(tensor=v.tensor, offset=v.offset + offset_delta, ap=ap_list)


def _r(ap):
    """Placeholder for fp32->float32r streaming (needs producer-side
    rounding on HW; disabled)."""
    return ap


def _body(nc, hist_in, cand_in, mask_in, pos_in, pos_emb, w1t, pos_embT,
          b1, w2, ur_out, cand_out, tc):
    import contextlib

    ctx = contextlib.ExitStack()
    with ctx:
        consts = ctx.enter_context(tc.tile_pool(name="consts", bufs=1))
        ps = ctx.enter_context(tc.tile_pool(name="ps", bufs=4, space="PSUM"))
        psm = ctx.enter_context(tc.tile_pool(name="psm", bufs=2, space="PSUM"))
        hidp = ctx.enter_context(tc.tile_pool(name="hid", bufs=3))
        smp = ctx.enter_context(tc.tile_pool(name="smp", bufs=3))
        amcp = ctx.enter_context(tc.tile_pool(name="amcp", bufs=2))
        eTp = ctx.enter_context(tc.tile_pool(name="eTp", bufs=3))

        # ---------------- constants ----------------
        ident = consts.tile([128, 128], DT)
        make_identity(nc, ident)
        identb = consts.tile([128, 128], BF)
        nc.scalar.copy(out=identb, in_=ident)

        # W1T[f, a] in 10 f-chunks of 100 (host provides W1 transposed, bf16)
        w1T = consts.tile([100, 10, A], BF)
        nc.sync.dma_start(out=w1T,
                          in_=w1t.ap().rearrange("(k p) a -> p k a", p=100))

        pos_emb_s = consts.tile([J, P], BF)
        nc.gpsimd.dma_start(out=pos_emb_s, in_=pos_emb.ap())
        posT = consts.tile([P, J], BF)
        nc.gpsimd.dma_start(out=posT, in_=pos_embT.ap())

        # E[j, a] = pos_emb @ Wh2.T  (Wh2 = W1[:, 900:1000])
        E_s = consts.tile([J, A], BF)
        psE = ps.tile([J, A], DT, tag="ps")
        nc.tensor.matmul(psE, lhsT=posT[:, :], rhs=w1T[:, 9, :],
                         start=True, stop=True)
        nc.vector.tensor_copy(out=E_s, in_=psE)

        b1row = consts.tile([1, A], DT)
        nc.sync.dma_start(out=b1row, in_=b1.ap())
        one11 = consts.tile([1, 1], DT)
        nc.vector.memset(one11, 1.0)

        # c0[a] = Wc2 @ pos_emb[1] + b1 as two per-partition bias columns
        c0col = consts.tile([100, 2], DT)
        for ac in range(2):
            asl = slice(ac * 100, (ac + 1) * 100)
            psc = ps.tile([100, 1], DT, tag="ps")
            nc.tensor.matmul(psc, lhsT=_r(w1T[:, 4, asl]), rhs=_r(posT[:, 1:2]),
                             start=True, stop=False)
            nc.tensor.matmul(psc, lhsT=_r(b1row[:, asl]), rhs=_r(one11[:, :]),
                             start=False, stop=True)
            nc.scalar.copy(out=c0col[:, ac:ac + 1], in_=psc)

        w2col = consts.tile([100, 2], HDT)
        nc.gpsimd.dma_start(out=w2col,
                            in_=w2.ap().rearrange("(c p) -> p c", p=100))

        # mask bias (mask-1)*1e9 in [c, q, b, h] layout (broadcast over c, q)
        mb25 = consts.tile([NC2, 2, BC, H], DT)
        nc.gpsimd.dma_start(out=mb25, in_=_bc(_bc(mask_in.ap(), 0, 2), 0, NC2))
        nc.scalar.activation(out=mb25, in_=mb25, func=AF.Copy,
                             bias=-1e9, scale=1e9)

        # one-hot of positions, transposed: onehot[j, b*H+h] = (pos[b,h]==j)
        pos52 = consts.tile([J, BC * H], I32)
        nc.gpsimd.dma_start(out=pos52, in_=_bc(pos_in.ap(), 0, J))
        iot = consts.tile([J, BC * H], I32)
        nc.gpsimd.iota(iot, pattern=[[0, BC * H]], base=0, channel_multiplier=1)
        onehot_s = consts.tile([J, BC * H], BF)
        nc.vector.tensor_tensor(out=onehot_s, in0=iot, in1=pos52, op=ALU.is_equal)

        # ---------------- data load + transpose ----------------
        # bf16 cast-loads (gpsimd-initiated): halves SBUF traffic and makes
        # the PE transposes 1 cycle/row
        cand_all = consts.tile([100, 4, D], BF)   # [2x50 rows, batch-pair, feat]
        hist_all = consts.tile([100, 4, D], BF)
        for hf in range(2):
            sl = slice(hf * 50, (hf + 1) * 50)
            src_c = _ap(cand_in.ap(), hf * N * D,
                        [[D, 50], [2 * N * D, 4], [1, D]])
            src_h = _ap(hist_in.ap(), hf * H * D,
                        [[D, 50], [2 * H * D, 4], [1, D]])
            nc.gpsimd.dma_start(out=cand_all[sl, :, :], in_=src_c)
            nc.gpsimd.dma_start(out=hist_all[sl, :, :], in_=src_h)

        candT = consts.tile([100, 4, BC * N], BF)  # [feat-chunk, k, (b,n)]
        histT = consts.tile([100, 4, BC * H], BF)
        for g in range(4 if "transp" not in SKIP else 0):
            ptc = ps.tile([100, 4, 100], BF, tag="ps")
            pth = ps.tile([100, 4, 100], BF, tag="ps")
            for k in range(4):
                nc.tensor.transpose(
                    ptc[:, k, :],
                    cand_all[:, g, k * 100:(k + 1) * 100],
                    identb[:100, :100])
                nc.tensor.transpose(
                    pth[:, k, :],
                    hist_all[:, g, k * 100:(k + 1) * 100],
                    identb[:100, :100])
            nc.scalar.copy(out=candT[:, :, g * 100:(g + 1) * 100], in_=ptc)
            nc.scalar.copy(out=histT[:, :, g * 100:(g + 1) * 100], in_=pth)

        # hist with position columns, natural layout, all batches (bf16 for
        # the final attention matmul; fp32 staging for the DMA'd D part)
        histf_b = consts.tile([H, BC, F], BF)
        src_hf = _ap(hist_in.ap(), 0, [[D, H], [H * D, BC], [1, D]])
        nc.gpsimd.dma_start(out=histf_b[:, :, 0:D], in_=src_hf)
        for b in range(BC):
            ppg = ps.tile([H, P], DT, tag="ps")
            nc.tensor.matmul(ppg, lhsT=onehot_s[:, b * H:(b + 1) * H],
                             rhs=pos_emb_s[:, :], start=True, stop=True)
            nc.scalar.copy(out=histf_b[:, b, D:F], in_=ppg)

        # candidate passthrough: DRAM->DRAM strided copy + pos_emb[1] bcast
        dst_c = _ap(cand_out.ap(), 0, [[F, BC * N], [1, D]])
        src_c0 = _ap(cand_in.ap(), 0, [[D, BC * N], [1, D]])
        nc.sync.dma_start(out=dst_c, in_=src_c0)
        nc.gpsimd.dma_start(
            out=cand_out.ap()[:, :, D:F],
            in_=_bc(_bc(pos_emb.ap()[1:2, :], 0, N), 0, BC))

        # ------- GEMMs: hcT2[a, (b,n), dup2] (duplicated), hhT[a, (b,h)] ----
        hcT2 = consts.tile([100, 2, BC * N, 2], HDT)
        hhT = consts.tile([100, 2, BC * H], HDT)
        # batch-halved gemm chunks so batch 0's hidden starts sooner
        for ac in range(2 if "gemm" not in SKIP else 0):
            asl = slice(ac * 100, (ac + 1) * 100)
            for half in range(2):
                csl = slice(half * BC * N // 2, (half + 1) * BC * N // 2)
                hsl = slice(half * BC * H // 2, (half + 1) * BC * H // 2)
                pg = ps.tile([100, BC * N // 2], DT, tag="ps")
                for k in range(4):
                    nc.tensor.matmul(pg, lhsT=_r(w1T[:, k, asl]),
                                     rhs=_r(candT[:, k, csl]),
                                     start=(k == 0), stop=(k == 3))
                # evacuate + add c0 bias, duplicating each element (dup2)
                nc.scalar.activation(out=hcT2[:, ac, csl, :],
                                     in_=_bc(pg[:, :], 2, 2),
                                     func=AF.Identity,
                                     bias=c0col[:, ac:ac + 1], scale=1.0)

                ph = ps.tile([100, BC * H // 2], DT, tag="ps")
                for k in range(4):
                    nc.tensor.matmul(ph, lhsT=_r(w1T[:, 5 + k, asl]),
                                     rhs=_r(histT[:, k, hsl]),
                                     start=(k == 0), stop=False)
                nc.tensor.matmul(ph, lhsT=_r(E_s[:, asl]),
                                 rhs=_r(onehot_s[:, hsl]),
                                 start=False, stop=True)
                # store NEGATED hh: hidden is rebuilt as max(hc, -hh); the
                # dropped +hh term is folded into the mask-bias tile below
                nc.scalar.activation(out=hhT[:, ac, hsl], in_=ph,
                                     func=AF.Copy, scale=-1.0)

        # S[(b,h)] = sum_a w2[a]*(-hh[a,(b,h)]); mbc = mb25 - S  (so the
        # softmax logits get +sum_a w2*hh added back, batched once)
        w2c25 = consts.tile([100, 2, NC2], HDT)
        for ac in range(2):
            nc.vector.tensor_copy(out=w2c25[:, ac, :],
                                  in_=_bc(w2col[:, ac:ac + 1], 1, NC2))
        psS = ps.tile([NC2, BC * H], DT, tag="ps")
        for ac in range(2):
            nc.tensor.matmul(psS, lhsT=w2c25[:, ac, :],
                             rhs=hhT[:, ac, :],
                             start=(ac == 0), stop=(ac == 1))
        mbc = consts.tile([NC2, 2, BC, H], DT)
        Ssb = consts.tile([NC2, BC * H], DT)
        nc.scalar.copy(out=Ssb, in_=psS)
        Ssb_b = _ap(Ssb[:, :], 0,
                    [list(Ssb[:, :].ap[0]), [0, 2], [H, BC], [1, H]])
        nc.gpsimd.tensor_tensor(out=mbc, in0=mb25, in1=Ssb_b,
                                op=ALU.subtract)

        # ---- per-batch fused pipeline: hidden max -> matvec -> softmax ->
        # ---- attention. No cross-batch barrier; engines pipeline batches.
        urs_all = consts.tile([NC2, 2, BC, F], BF)
        for b in range(BC):
            hids = []
            for ac in range(2):
                # hc broadcast over h via duplicated pairs: free = (n, hq, hr)
                v = hcT2[:, ac, b * N:(b + 1) * N, :]   # [[p],[2,50],[1,2]]
                hcb = _bc(v, 2, H // 2)                 # [100, 50, 25, 2]
                w = hhT[:, ac, b * H:(b + 1) * H]       # [[p],[1,50]]
                hhb = _ap(w, 0, [list(w.ap[0]), [0, N], [2, H // 2], [1, 2]])
                hid = hidp.tile([100, N, H // 2, 2], HDT, tag=f"hid{ac}")
                if "hidden" not in SKIP:
                    # DVE-only: gpsimd has no ucode for TT max, ACT can't TT
                    nc.vector.tensor_tensor(out=hid, in0=hcb, in1=hhb,
                                            op=ALU.max)
                hids.append(hid)
            amc = psm.tile([100, NC2], DT, tag="amc")
            for c in range(NC2 if "matvec" not in SKIP else 0):
                for ac in range(2):
                    hv = hids[ac]
                    pst = [list(x) for x in hv.ap][0]
                    lhs = _ap(hv, c * 100, [pst, [1, 100]])
                    nc.tensor.matmul(amc[:, c:c + 1], lhsT=_r(lhs),
                                     rhs=_r(w2col[:, ac:ac + 1]),
                                     start=(ac == 0), stop=(ac == 1))
            if "matvec" in SKIP:
                continue
            amcs = amcp.tile([100, NC2], DT, tag="amcs")
            nc.scalar.copy(out=amcs, in_=amc)
            amT = psm.tile([NC2, 2 * H], DT, tag="amT")
            nc.tensor.transpose(amT[:, :], _r(amcs[:, :]),
                                _r(ident[:100, :100]))
            # evacuate logits on ACT, then mask-add on gpsimd (SBUF only)
            amrb = smp.tile([NC2, 2, H], DT, tag="amr")
            amT_r = _ap(amT[:, :], 0, [list(amT[:, :].ap[0]), [H, 2], [1, H]])
            nc.scalar.copy(out=amrb, in_=amT_r)
            ammb = smp.tile([NC2, 2, H], DT, tag="amm")
            nc.gpsimd.tensor_add(out=ammb, in0=amrb, in1=mbc[:, :, b, :])
            nmb = smp.tile([NC2, 2], DT, tag="nm")
            nc.vector.tensor_reduce(out=nmb, in_=ammb, axis=AX.X, op=ALU.max,
                                    negate=True)
            am2b = smp.tile([NC2, 2, H], DT, tag="am2")
            nc.gpsimd.tensor_add(out=am2b, in0=ammb, in1=_bc(nmb[:, :], 2, H))
            exb = smp.tile([NC2, 2, H], BF, tag="ex")
            nc.scalar.activation(out=exb, in_=am2b, func=AF.Exp)
            ssumb = smp.tile([NC2, 2], DT, tag="ssum")
            nc.vector.tensor_reduce(out=ssumb, in_=exb, axis=AX.X, op=ALU.add)
            rsb = smp.tile([NC2, 2], DT, tag="rs")
            nc.vector.reciprocal(rsb, ssumb)

            if "final" in SKIP:
                continue
            # both parities transposed into one PSUM tile (q1 at partition
            # 64 — PE tiles must start at 0/32/64/96), single evacuation
            for q in range(2):
                peT = ps.tile([H, NC2], BF, tag="ps")
                nc.tensor.transpose(peT[:, :], exb[:, q, :],
                                    identb[:NC2, :NC2])
                eT = eTp.tile([H, NC2], BF, tag="eT")
                nc.scalar.copy(out=eT, in_=peT)
                pur = ps.tile([NC2, F], DT, tag="ps")
                nc.tensor.matmul(pur, lhsT=eT[:, :],
                                 rhs=histf_b[:, b, :],
                                 start=True, stop=True)
                # normalize during PSUM evacuation (gpsimd cannot see PSUM)
                nc.scalar.activation(out=urs_all[:, q, b, :], in_=pur,
                                     func=AF.Copy, scale=rsb[:, q:q + 1])
            # stream this batch's slice out immediately (overlaps compute)
            dst_b = _ap(ur_out.ap(), b * N * F,
                        [[2 * F, NC2], [F, 2], [1, F]])
            nc.sync.dma_start(out=dst_b, in_=urs_all[:, :, b, :])



def build(debug=False, reps=1):
    nc = bacc.Bacc("TRN2", target_bir_lowering=False, debug=debug)
    hist_in = nc.dram_tensor("hist_in", [BC, H, D], DT, kind="ExternalInput")
    cand_in = nc.dram_tensor("cand_in", [BC, N, D], DT, kind="ExternalInput")
    mask_in = nc.dram_tensor("mask_in", [BC, H], DT, kind="ExternalInput")
    pos_in = nc.dram_tensor("pos_in", [BC, H], I32, kind="ExternalInput")
    pos_emb = nc.dram_tensor("pos_emb", [J, P], DT, kind="ExternalInput")
    w1t = nc.dram_tensor("w1t", [2 * F, A], BF, kind="ExternalInput")
    pos_embT = nc.dram_tensor("pos_embT", [P, J], DT, kind="ExternalInput")
    b1 = nc.dram_tensor("b1", [A], DT, kind="ExternalInput")
    w2 = nc.dram_tensor("w2", [A], DT, kind="ExternalInput")
    ur_out = nc.dram_tensor("ur_out", [BC, N, F], BF, kind="ExternalOutput")
    cand_out = nc.dram_tensor("cand_out", [BC, N, F], DT, kind="ExternalOutput")

    with tile.TileContext(nc) as tc:
        for _ in range(reps):
            _body(nc, hist_in, cand_in, mask_in, pos_in, pos_emb, w1t,
                  pos_embT, b1, w2, ur_out, cand_out, tc)
    nc.compile()
    return nc


_NC = None


def _get_nc():
    global _NC
    if _NC is None:
        _NC = build(debug=False)
    return _NC


def make_in_maps(history_repr, candidate_repr, user_history_mask,
                 user_history_position, pos_emb, W1, b1, w2):
    hist = np.ascontiguousarray(np.asarray(history_repr, np.float32))
    cand = np.ascontiguousarray(np.asarray(candidate_repr, np.float32))
    mask = np.asarray(user_history_mask).astype(np.float32)
    pos = np.asarray(user_history_position).astype(np.int32)
    pe = np.ascontiguousarray(np.asarray(pos_emb, np.float32))
    import ml_dtypes
    w1t = np.ascontiguousarray(
        np.asarray(W1, np.float32).T.astype(ml_dtypes.bfloat16))
    peT = np.ascontiguousarray(pe.T)
    b1_ = np.ascontiguousarray(np.asarray(b1, np.float32))
    w2_ = np.ascontiguousarray(np.asarray(w2, np.float32))
    in_maps = []
    for c in range(NCORES):
        sl = slice(c * BC, (c + 1) * BC)
        in_maps.append({
            "hist_in": hist[sl], "cand_in": cand[sl],
            "mask_in": mask[sl], "pos_in": pos[sl],
            "pos_emb": pe, "w1t": w1t, "pos_embT": peT,
            "b1": b1_, "w2": w2_,
        })
    return in_maps


def kernel(history_repr, candidate_repr, user_history_mask,
           user_history_position, pos_emb, W1, b1, w2, b2=None, **_ignored):
    # b2 shifts every logit equally -> cancels in softmax; unused.
    nc = _get_nc()
    in_maps = make_in_maps(history_repr, candidate_repr, user_history_mask,
                           user_history_position, pos_emb, W1, b1, w2)
    res = bass_utils.run_bass_kernel_spmd(nc, in_maps, list(range(NCORES)))
    ur = np.concatenate(
        [np.asarray(res.results[c]["ur_out"]).astype(np.float32)
         for c in range(NCORES)], 0)
    cand = np.concatenate([res.results[c]["cand_out"] for c in range(NCORES)], 0)
    return ur, cand



# revision 79
# speedup vs baseline: 2.7914x; 1.2099x over previous
"""Trainium2 Bass kernel for the news-attention module.

Computes, per batch b:
    hist = [history_repr | pos_emb[positions]]            [H, 500]
    cand = [candidate_repr | pos_emb[1]]                  [N, 500]
    hc = cand @ Wc.T ; hh = hist @ Wh.T                   [*, 200]
    a[n,h] = w2 . relu(hc[n] + hh[h] + b1)
    alpha = softmax_h(mask ? a : -1e9)
    out1 = alpha @ hist ; out2 = cand

Structure:
  - position gather folded into matmuls: pos part of hh = onehot(pos) @ E
    with E = pos_emb @ Wh2.T; candidate pos part + b1 folded into a
    per-partition bias column c0 applied during PSUM evacuation.
  - all fp32 matmuls stream as float32r (full-rate fp32 mode for moving
    free dim >= 256); psum accumulation is fp32 either way.
  - hidden built in [A-chunk, (n, h)] layout with zero-stride broadcast
    APs on the DVE; hc stored with each element duplicated (pairs) so the
    innermost AP dim is packed, enabling the DVE 2-byte fast path in bf16
    mode; relu in place.
  - w2 contraction in column form: lhsT = hidden chunk (100 pairs),
    rhs = w2 column, accumulating into a [100, 25] psum per batch ->
    logits leave PSUM via one cheap 100-lane copy instead of 40
    single-lane row copies.
  - softmax over h batched for all batches in [25, parity, b, h] layout
    (n = 2c + parity); mask bias tile built in the same layout.
  - alpha @ hist as two parity matmuls per batch with 1/sum folded into
    the PSUM-evacuation scale.

Sharding: data-parallel over batch, 8 batches per core on 8 cores.
Params replicated. Full inputs in, full outputs out.
"""

import sys

for _p in ("/opt/trn_rl_repo",):
    if _p not in sys.path:
        sys.path.insert(0, _p)

import numpy as np

import concourse.bass as bass
import concourse.bacc as bacc
import concourse.tile as tile
from concourse import mybir
from concourse import bass_utils
from concourse.masks import make_identity

DT = mybir.dt.float32
FR = mybir.dt.float32r
BF = mybir.dt.bfloat16
I32 = mybir.dt.int32
AF = mybir.ActivationFunctionType
ALU = mybir.AluOpType
AX = mybir.AxisListType

NCORES = 8
B = 64
BC = B // NCORES  # 8 batches per core
H = 50
N = 50
D = 400
P = 100
A = 200
F = D + P       # 500
J = 52
NC2 = N // 2    # 25 pair-chunks of 100 pairs (2 candidates) per batch

# bf16 hidden pipeline (hc/hh/hidden/w2 in bf16; everything else fp32)
USE_BF16 = True
SKIP = set()  # timing ablations: {"hidden","matvec","gemm","transp","final"}
MATVEC_ROW = False  # row-form matvec (fewer, wider PE ops + ACT evac)
HDT = BF if USE_BF16 else DT


def _bc(v, pos, n):
    """Insert a zero-stride (broadcast) dim of length n at position pos."""
    ap = [list(x) for x in v.ap]
    ap.insert(pos, [0, n])
    return bass.AP(tensor=v.tensor, offset=v.offset, ap=ap)


def _ap(v, offset_delta, ap_list):
    return bass.AP(tensor=v.tensor, offset=v.offset + offset_delta, ap=ap_list)


def _r(ap):
    """Placeholder for fp32->float32r streaming (needs producer-side
    rounding on HW; disabled)."""
    return ap


def _body(nc, hist_b_in, cand_b_in, mask_in, pos_in, pos_emb, w1t,
          pos_embT, b1, w2, ur_out, cand_out, tc):
    import contextlib

    ctx = contextlib.ExitStack()
    with ctx:
        consts = ctx.enter_context(tc.tile_pool(name="consts", bufs=1))
        ps = ctx.enter_context(tc.tile_pool(name="ps", bufs=4, space="PSUM"))
        psm = ctx.enter_context(tc.tile_pool(name="psm", bufs=2, space="PSUM"))
        hidp = ctx.enter_context(tc.tile_pool(name="hid", bufs=3))
        smp = ctx.enter_context(tc.tile_pool(name="smp", bufs=3))
        amcp = ctx.enter_context(tc.tile_pool(name="amcp", bufs=2))
        eTp = ctx.enter_context(tc.tile_pool(name="eTp", bufs=3))

        # ------- input loads: all on the SP HW queue in consumption order
        # (ACT-queue DMAs would block ACT compute; gpsimd gets the small
        # casts and the late histf load)
        cand_all = consts.tile([100, 4, D], BF)  # [2x50 rows, batch-pair, feat]
        hist_all = consts.tile([100, 4, D], BF)
        w1T = consts.tile([100, 10, A], BF)
        histf_b = consts.tile([H, BC, F], BF)
        # SP queue: cand (first need), then W1T halves, then passthrough
        for hf in range(2):
            sl = slice(hf * 50, (hf + 1) * 50)
            src_c = _ap(cand_b_in.ap(), hf * N * D,
                        [[D, 50], [2 * N * D, 4], [1, D]])
            nc.sync.dma_start(out=cand_all[sl, :, :], in_=src_c)
        nc.sync.dma_start(
            out=w1T[:, 0:5, :],
            in_=w1t.ap()[0:5 * 100, :].rearrange("(k p) a -> p k a", p=100))
        nc.sync.dma_start(
            out=w1T[:, 5:10, :],
            in_=w1t.ap()[5 * 100:, :].rearrange("(k p) a -> p k a", p=100))
        # candidate passthrough: DRAM->DRAM strided copy (bf16 out)
        dst_c = _ap(cand_out.ap(), 0, [[F, BC * N], [1, D]])
        src_c0 = _ap(cand_b_in.ap(), 0, [[D, BC * N], [1, D]])
        nc.sync.dma_start(out=dst_c, in_=src_c0)

        # ---------------- constants ----------------
        # bf16 identity first (gates the input transposes); fp32 derived
        identb = consts.tile([128, 128], BF)
        make_identity(nc, identb)
        ident = consts.tile([128, 128], DT)
        nc.vector.tensor_copy(out=ident, in_=identb)

        pos_emb_s = consts.tile([J, P], BF)
        nc.gpsimd.dma_start(out=pos_emb_s, in_=pos_emb.ap())
        posT = consts.tile([P, J], BF)
        nc.gpsimd.dma_start(out=posT, in_=pos_embT.ap())
        # hist staging rides the gpsimd queue (SP is full with cand+W1T)
        for hf in range(2):
            sl = slice(hf * 50, (hf + 1) * 50)
            src_h = _ap(hist_b_in.ap(), hf * H * D,
                        [[D, 50], [2 * H * D, 4], [1, D]])
            nc.gpsimd.dma_start(out=hist_all[sl, :, :], in_=src_h)

        # E[j, a] = pos_emb @ Wh2.T  (Wh2 = W1[:, 900:1000])
        E_s = consts.tile([J, A], BF)
        psE = ps.tile([J, A], DT, tag="ps")
        nc.tensor.matmul(psE, lhsT=posT[:, :], rhs=w1T[:, 9, :],
                         start=True, stop=True)
        nc.vector.tensor_copy(out=E_s, in_=psE)

        b1row = consts.tile([1, A], DT)
        nc.sync.dma_start(out=b1row, in_=b1.ap())
        one11 = consts.tile([1, 1], DT)
        nc.vector.memset(one11, 1.0)

        # c0[a] = Wc2 @ pos_emb[1] + b1 as two per-partition bias columns
        c0col = consts.tile([100, 2], DT)
        for ac in range(2):
            asl = slice(ac * 100, (ac + 1) * 100)
            psc = ps.tile([100, 1], DT, tag="ps")
            nc.tensor.matmul(psc, lhsT=_r(w1T[:, 4, asl]), rhs=_r(posT[:, 1:2]),
                             start=True, stop=False)
            nc.tensor.matmul(psc, lhsT=_r(b1row[:, asl]), rhs=_r(one11[:, :]),
                             start=False, stop=True)
            nc.scalar.copy(out=c0col[:, ac:ac + 1], in_=psc)

        w2col = consts.tile([100, 2], HDT)
        nc.gpsimd.dma_start(out=w2col,
                            in_=w2.ap().rearrange("(c p) -> p c", p=100))

        # mask bias (mask-1)*1e9 in [c, q, b, h] layout (broadcast over c, q)
        mb25 = consts.tile([NC2, 2, BC, H], DT)
        nc.gpsimd.dma_start(out=mb25, in_=_bc(_bc(mask_in.ap(), 0, 2), 0, NC2))
        nc.scalar.activation(out=mb25, in_=mb25, func=AF.Copy,
                             bias=-1e9, scale=1e9)

        # one-hot of positions, transposed: onehot[j, b*H+h] = (pos[b,h]==j)
        pos52 = consts.tile([J, BC * H], I32)
        nc.gpsimd.dma_start(out=pos52, in_=_bc(pos_in.ap(), 0, J))
        iot = consts.tile([J, BC * H], I32)
        nc.gpsimd.iota(iot, pattern=[[0, BC * H]], base=0, channel_multiplier=1)
        onehot_s = consts.tile([J, BC * H], BF)
        nc.vector.tensor_tensor(out=onehot_s, in0=iot, in1=pos52, op=ALU.is_equal)

        # history in natural layout for the final attention — late need, so
        # last on the gpsimd queue
        src_hf = _ap(hist_b_in.ap(), 0, [[D, H], [H * D, BC], [1, D]])
        nc.gpsimd.dma_start(out=histf_b[:, :, 0:D], in_=src_hf)

        # ---------------- transposes ----------------
        candT = consts.tile([100, 4, BC * N], BF)  # [feat-chunk, k, (b,n)]
        histT = consts.tile([100, 4, BC * H], BF)
        for g in range(4 if "transp" not in SKIP else 0):
            ptc = ps.tile([100, 4, 100], BF, tag="ps")
            pth = ps.tile([100, 4, 100], BF, tag="ps")
            for k in range(4):
                nc.tensor.transpose(
                    ptc[:, k, :],
                    cand_all[:, g, k * 100:(k + 1) * 100],
                    identb[:100, :100])
                nc.tensor.transpose(
                    pth[:, k, :],
                    hist_all[:, g, k * 100:(k + 1) * 100],
                    identb[:100, :100])
            nc.scalar.copy(out=candT[:, :, g * 100:(g + 1) * 100], in_=ptc)
            nc.vector.tensor_copy(out=histT[:, :, g * 100:(g + 1) * 100],
                                  in_=pth)

        # candidate pos_emb[1] broadcast fill (fp32 -> bf16 cast on gpsimd)
        nc.gpsimd.dma_start(
            out=cand_out.ap()[:, :, D:F],
            in_=_bc(_bc(pos_emb.ap()[1:2, :], 0, N), 0, BC))

        # ------- GEMMs: hcT2[a, (b,n), dup2] (duplicated), hhT[a, (b,h)] ----
        hcT2 = consts.tile([100, 2, BC * N, 2], HDT)
        hhT = consts.tile([100, 2, BC * H], HDT)
        # batch-halved gemm chunks so batch 0's hidden starts sooner
        for ac in range(2 if "gemm" not in SKIP else 0):
            asl = slice(ac * 100, (ac + 1) * 100)
            for half in range(2):
                csl = slice(half * BC * N // 2, (half + 1) * BC * N // 2)
                hsl = slice(half * BC * H // 2, (half + 1) * BC * H // 2)
                pg = ps.tile([100, BC * N // 2], DT, tag="ps")
                for k in range(4):
                    nc.tensor.matmul(pg, lhsT=_r(w1T[:, k, asl]),
                                     rhs=_r(candT[:, k, csl]),
                                     start=(k == 0), stop=(k == 3))
                # evacuate + add c0 bias, duplicating each element (dup2)
                nc.scalar.activation(out=hcT2[:, ac, csl, :],
                                     in_=_bc(pg[:, :], 2, 2),
                                     func=AF.Identity,
                                     bias=c0col[:, ac:ac + 1], scale=1.0)

                ph = ps.tile([100, BC * H // 2], DT, tag="ps")
                for k in range(4):
                    nc.tensor.matmul(ph, lhsT=_r(w1T[:, 5 + k, asl]),
                                     rhs=_r(histT[:, k, hsl]),
                                     start=(k == 0), stop=False)
                nc.tensor.matmul(ph, lhsT=_r(E_s[:, asl]),
                                 rhs=_r(onehot_s[:, hsl]),
                                 start=False, stop=True)
                # store NEGATED hh: hidden is rebuilt as max(hc, -hh); the
                # dropped +hh term is folded into the mask-bias tile below
                nc.scalar.activation(out=hhT[:, ac, hsl], in_=ph,
                                     func=AF.Copy, scale=-1.0)

        # S[(b,h)] = sum_a w2[a]*(-hh[a,(b,h)]); mbc = mb25 - S  (so the
        # softmax logits get +sum_a w2*hh added back, batched once)
        w2c25 = consts.tile([100, 2, NC2], HDT)
        for ac in range(2):
            nc.vector.tensor_copy(out=w2c25[:, ac, :],
                                  in_=_bc(w2col[:, ac:ac + 1], 1, NC2))
        psS = ps.tile([NC2, BC * H], DT, tag="ps")
        for ac in range(2):
            nc.tensor.matmul(psS, lhsT=w2c25[:, ac, :],
                             rhs=hhT[:, ac, :],
                             start=(ac == 0), stop=(ac == 1))
        mbc = consts.tile([NC2, 2, BC, H], DT)
        Ssb = consts.tile([NC2, BC * H], DT)
        nc.scalar.copy(out=Ssb, in_=psS)
        Ssb_b = _ap(Ssb[:, :], 0,
                    [list(Ssb[:, :].ap[0]), [0, 2], [H, BC], [1, H]])
        nc.gpsimd.tensor_tensor(out=mbc, in0=mb25, in1=Ssb_b,
                                op=ALU.subtract)

        # hist position columns gathered via one-hot matmuls (feeds the
        # final attention only — deprioritized behind the gemm evacs)
        for b in range(BC):
            ppg = ps.tile([H, P], DT, tag="ps")
            nc.tensor.matmul(ppg, lhsT=onehot_s[:, b * H:(b + 1) * H],
                             rhs=pos_emb_s[:, :], start=True, stop=True)
            nc.scalar.copy(out=histf_b[:, b, D:F], in_=ppg)

        # ---- per-batch fused pipeline: hidden max -> matvec -> softmax ->
        # ---- attention. No cross-batch barrier; engines pipeline batches.
        urs_all = consts.tile([NC2, 2, BC, F], BF)
        for b in range(BC):
            hids = []
            for ac in range(2):
                # hc broadcast over h via duplicated pairs: free = (n, hq, hr)
                v = hcT2[:, ac, b * N:(b + 1) * N, :]   # [[p],[2,50],[1,2]]
                hcb = _bc(v, 2, H // 2)                 # [100, 50, 25, 2]
                w = hhT[:, ac, b * H:(b + 1) * H]       # [[p],[1,50]]
                hhb = _ap(w, 0, [list(w.ap[0]), [0, N], [2, H // 2], [1, 2]])
                hid = hidp.tile([100, N, H // 2, 2], HDT, tag=f"hid{ac}")
                if "hidden" not in SKIP:
                    # DVE-only: gpsimd has no ucode for TT max, ACT can't TT
                    nc.vector.tensor_tensor(out=hid, in0=hcb, in1=hhb,
                                            op=ALU.max)
                hids.append(hid)
            amc = psm.tile([100, NC2], DT, tag="amc")
            for c in range(NC2 if "matvec" not in SKIP else 0):
                for ac in range(2):
                    hv = hids[ac]
                    pst = [list(x) for x in hv.ap][0]
                    lhs = _ap(hv, c * 100, [pst, [1, 100]])
                    nc.tensor.matmul(amc[:, c:c + 1], lhsT=_r(lhs),
                                     rhs=_r(w2col[:, ac:ac + 1]),
                                     start=(ac == 0), stop=(ac == 1))
            if "matvec" in SKIP:
                continue
            amcs = amcp.tile([100, NC2], DT, tag="amcs")
            nc.scalar.copy(out=amcs, in_=amc)
            amT = psm.tile([NC2, 2 * H], DT, tag="amT")
            nc.tensor.transpose(amT[:, :], _r(amcs[:, :]),
                                _r(ident[:100, :100]))
            # evacuate logits on ACT, then mask-add on gpsimd (SBUF only).
            # logits are O(10), far from exp overflow, so no max-subtract;
            # exp and its row-sum fuse into one ACT op per parity.
            amrb = smp.tile([NC2, 2, H], DT, tag="amr")
            amT_r = _ap(amT[:, :], 0, [list(amT[:, :].ap[0]), [H, 2], [1, H]])
            nc.scalar.copy(out=amrb, in_=amT_r)
            ammb = smp.tile([NC2, 2, H], DT, tag="amm")
            nc.gpsimd.tensor_add(out=ammb, in0=amrb, in1=mbc[:, :, b, :])
            exb = smp.tile([NC2, 2, H], BF, tag="ex")
            ssumb = smp.tile([NC2, 2], DT, tag="ssum")
            for q in range(2):
                nc.scalar.activation(out=exb[:, q, :], in_=ammb[:, q, :],
                                     func=AF.Exp,
                                     accum_out=ssumb[:, q:q + 1])
            rsb = smp.tile([NC2, 2], DT, tag="rs")
            nc.vector.reciprocal(rsb, ssumb)

            if "final" in SKIP:
                continue
            # both parities transposed into one PSUM tile (q1 at partition
            # 64 — PE tiles must start at 0/32/64/96), single evacuation
            for q in range(2):
                peT = ps.tile([H, NC2], BF, tag="ps")
                nc.tensor.transpose(peT[:, :], exb[:, q, :],
                                    identb[:NC2, :NC2])
                eT = eTp.tile([H, NC2], BF, tag="eT")
                nc.scalar.copy(out=eT, in_=peT)
                pur = ps.tile([NC2, F], DT, tag="ps")
                nc.tensor.matmul(pur, lhsT=eT[:, :],
                                 rhs=histf_b[:, b, :],
                                 start=True, stop=True)
                # normalize during PSUM evacuation (gpsimd cannot see PSUM);
                # DVE helps only on the last batches, after its max streak
                if q == 1 and b >= BC - 2:
                    nc.vector.tensor_scalar_mul(out=urs_all[:, q, b, :],
                                                in0=pur,
                                                scalar1=rsb[:, q:q + 1])
                else:
                    nc.scalar.activation(out=urs_all[:, q, b, :], in_=pur,
                                         func=AF.Copy, scale=rsb[:, q:q + 1])
            # stream this batch's slice out immediately (overlaps compute)
            dst_b = _ap(ur_out.ap(), b * N * F,
                        [[2 * F, NC2], [F, 2], [1, F]])
            nc.sync.dma_start(out=dst_b, in_=urs_all[:, :, b, :])



def build(debug=False, reps=1):
    nc = bacc.Bacc("TRN2", target_bir_lowering=False, debug=debug)
    hist_b_in = nc.dram_tensor("hist_b_in", [BC, H, D], BF,
                               kind="ExternalInput")
    cand_b_in = nc.dram_tensor("cand_b_in", [BC, N, D], BF,
                               kind="ExternalInput")
    mask_in = nc.dram_tensor("mask_in", [BC, H], DT, kind="ExternalInput")
    pos_in = nc.dram_tensor("pos_in", [BC, H], I32, kind="ExternalInput")
    pos_emb = nc.dram_tensor("pos_emb", [J, P], DT, kind="ExternalInput")
    w1t = nc.dram_tensor("w1t", [2 * F, A], BF, kind="ExternalInput")
    pos_embT = nc.dram_tensor("pos_embT", [P, J], DT, kind="ExternalInput")
    b1 = nc.dram_tensor("b1", [A], DT, kind="ExternalInput")
    w2 = nc.dram_tensor("w2", [A], DT, kind="ExternalInput")
    ur_out = nc.dram_tensor("ur_out", [BC, N, F], BF, kind="ExternalOutput")
    cand_out = nc.dram_tensor("cand_out", [BC, N, F], BF,
                              kind="ExternalOutput")

    with tile.TileContext(nc) as tc:
        for _ in range(reps):
            _body(nc, hist_b_in, cand_b_in, mask_in, pos_in,
                  pos_emb, w1t, pos_embT, b1, w2, ur_out, cand_out, tc)
    nc.compile()
    return nc


_NC = None


def _get_nc():
    global _NC
    if _NC is None:
        _NC = build(debug=False)
    return _NC


def make_in_maps(history_repr, candidate_repr, user_history_mask,
                 user_history_position, pos_emb, W1, b1, w2):
    import ml_dtypes
    hist = np.ascontiguousarray(np.asarray(history_repr, np.float32))
    cand = np.ascontiguousarray(np.asarray(candidate_repr, np.float32))
    hist_b = hist.astype(ml_dtypes.bfloat16)
    cand_b = cand.astype(ml_dtypes.bfloat16)
    mask = np.asarray(user_history_mask).astype(np.float32)
    pos = np.asarray(user_history_position).astype(np.int32)
    pe = np.ascontiguousarray(np.asarray(pos_emb, np.float32))
    import ml_dtypes
    w1t = np.ascontiguousarray(
        np.asarray(W1, np.float32).T.astype(ml_dtypes.bfloat16))
    peT = np.ascontiguousarray(pe.T)
    b1_ = np.ascontiguousarray(np.asarray(b1, np.float32))
    w2_ = np.ascontiguousarray(np.asarray(w2, np.float32))
    in_maps = []
    for c in range(NCORES):
        sl = slice(c * BC, (c + 1) * BC)
        in_maps.append({
            "hist_b_in": hist_b[sl], "cand_b_in": cand_b[sl],
            "mask_in": mask[sl], "pos_in": pos[sl],
            "pos_emb": pe, "w1t": w1t, "pos_embT": peT,
            "b1": b1_, "w2": w2_,
        })
    return in_maps


def kernel(history_repr, candidate_repr, user_history_mask,
           user_history_position, pos_emb, W1, b1, w2, b2=None, **_ignored):
    # b2 shifts every logit equally -> cancels in softmax; unused.
    nc = _get_nc()
    in_maps = make_in_maps(history_repr, candidate_repr, user_history_mask,
                           user_history_position, pos_emb, W1, b1, w2)
    res = bass_utils.run_bass_kernel_spmd(nc, in_maps, list(range(NCORES)))
    ur = np.concatenate(
        [np.asarray(res.results[c]["ur_out"]).astype(np.float32)
         for c in range(NCORES)], 0)
    cand = np.concatenate(
        [np.asarray(res.results[c]["cand_out"]).astype(np.float32)
         for c in range(NCORES)], 0)
    return ur, cand



# revision 87
# speedup vs baseline: 4.2325x; 1.5163x over previous
"""Trainium2 Bass kernel for the news-attention module.

Computes, per batch b:
    hist = [history_repr | pos_emb[positions]]            [H, 500]
    cand = [candidate_repr | pos_emb[1]]                  [N, 500]
    hc = cand @ Wc.T ; hh = hist @ Wh.T                   [*, 200]
    a[n,h] = w2 . relu(hc[n] + hh[h] + b1)
    alpha = softmax_h(mask ? a : -1e9)
    out1 = alpha @ hist ; out2 = cand

Structure:
  - position gather folded into matmuls: pos part of hh = onehot(pos) @ E
    with E = pos_emb @ Wh2.T; candidate pos part + b1 folded into a
    per-partition bias column c0 applied during PSUM evacuation.
  - all fp32 matmuls stream as float32r (full-rate fp32 mode for moving
    free dim >= 256); psum accumulation is fp32 either way.
  - hidden built in [A-chunk, (n, h)] layout with zero-stride broadcast
    APs on the DVE; hc stored with each element duplicated (pairs) so the
    innermost AP dim is packed, enabling the DVE 2-byte fast path in bf16
    mode; relu in place.
  - w2 contraction in column form: lhsT = hidden chunk (100 pairs),
    rhs = w2 column, accumulating into a [100, 25] psum per batch ->
    logits leave PSUM via one cheap 100-lane copy instead of 40
    single-lane row copies.
  - softmax over h batched for all batches in [25, parity, b, h] layout
    (n = 2c + parity); mask bias tile built in the same layout.
  - alpha @ hist as two parity matmuls per batch with 1/sum folded into
    the PSUM-evacuation scale.

Sharding: data-parallel over batch, 8 batches per core on 8 cores.
Params replicated. Full inputs in, full outputs out.
"""

import sys

for _p in ("/opt/trn_rl_repo",):
    if _p not in sys.path:
        sys.path.insert(0, _p)

import numpy as np

import concourse.bass as bass
import concourse.bacc as bacc
import concourse.tile as tile
from concourse import mybir
from concourse import bass_utils
from concourse.masks import make_identity

DT = mybir.dt.float32
FR = mybir.dt.float32r
BF = mybir.dt.bfloat16
I32 = mybir.dt.int32
AF = mybir.ActivationFunctionType
ALU = mybir.AluOpType
AX = mybir.AxisListType

NCORES = 8
B = 64
BC = B // NCORES  # 8 batches per core
H = 50
N = 50
D = 400
P = 100
A = 200
F = D + P       # 500
J = 52
NC2 = N // 2    # 25 pair-chunks of 100 pairs (2 candidates) per batch

# bf16 hidden pipeline (hc/hh/hidden/w2 in bf16; everything else fp32)
USE_BF16 = True
SKIP = set()  # timing ablations: {"hidden","matvec","gemm","transp","final"}
MATVEC_ROW = False  # row-form matvec (fewer, wider PE ops + ACT evac)
HDT = BF if USE_BF16 else DT


def _bc(v, pos, n):
    """Insert a zero-stride (broadcast) dim of length n at position pos."""
    ap = [list(x) for x in v.ap]
    ap.insert(pos, [0, n])
    return bass.AP(tensor=v.tensor, offset=v.offset, ap=ap)


def _ap(v, offset_delta, ap_list):
    return bass.AP(tensor=v.tensor, offset=v.offset + offset_delta, ap=ap_list)


def _r(ap):
    """Placeholder for fp32->float32r streaming (needs producer-side
    rounding on HW; disabled)."""
    return ap


def _body(nc, hist_b_in, cand_b_in, mask_in, pos_in, pos_emb, w1t,
          pos_embT, b1, w2, ur_out, cand_out, tc):
    import contextlib

    ctx = contextlib.ExitStack()
    with ctx:
        consts = ctx.enter_context(tc.tile_pool(name="consts", bufs=1))
        ps = ctx.enter_context(tc.tile_pool(name="ps", bufs=4, space="PSUM"))
        psm = ctx.enter_context(tc.tile_pool(name="psm", bufs=2, space="PSUM"))
        hidp = ctx.enter_context(tc.tile_pool(name="hid", bufs=3))
        smp = ctx.enter_context(tc.tile_pool(name="smp", bufs=3))
        amcp = ctx.enter_context(tc.tile_pool(name="amcp", bufs=2))
        eTp = ctx.enter_context(tc.tile_pool(name="eTp", bufs=3))

        # ------- input loads: all on the SP HW queue in consumption order
        # (ACT-queue DMAs would block ACT compute; gpsimd gets the small
        # casts and the late histf load)
        # separate tiles per chunk: dependency tracking for DMA-written
        # tiles is whole-tile, so writers must be 1:1 with consumer groups
        cand_all = consts.tile([100, 4, D], BF)  # [2x50 rows, batch-pair, feat]
        hist_all = consts.tile([100, 4, D], BF)
        w1T_c = consts.tile([100, 4, A], BF)   # cand-side W rows (k 0-3)
        w1T_h = consts.tile([100, 4, A], BF)   # hist-side W rows (k 5-8)
        w1T_4 = consts.tile([100, A], BF)      # cand pos rows (c0col)
        w1T_9 = consts.tile([100, A], BF)      # hist pos rows (E_s)
        histf_b = consts.tile([H, BC, F], BF)
        # SP queue: cand (first need), then W1T chunks, then passthrough
        for hf in range(2):
            sl = slice(hf * 50, (hf + 1) * 50)
            src_c = _ap(cand_b_in.ap(), hf * N * D,
                        [[D, 50], [2 * N * D, 4], [1, D]])
            nc.sync.dma_start(out=cand_all[sl, :, :], in_=src_c)
        for k, t in ((4, w1T_4), (9, w1T_9)):
            nc.sync.dma_start(out=t, in_=w1t.ap()[k * 100:(k + 1) * 100, :])
        nc.sync.dma_start(
            out=w1T_c,
            in_=w1t.ap()[0:400, :].rearrange("(k p) a -> p k a", p=100))
        nc.scalar.dma_start(
            out=w1T_h,
            in_=w1t.ap()[500:900, :].rearrange("(k p) a -> p k a", p=100))
        # candidate passthrough: DRAM->DRAM strided copy (bf16 out)
        dst_c = _ap(cand_out.ap(), 0, [[F, BC * N], [1, D]])
        src_c0 = _ap(cand_b_in.ap(), 0, [[D, BC * N], [1, D]])
        nc.sync.dma_start(out=dst_c, in_=src_c0)

        # ---------------- constants ----------------
        # bf16 identity first (gates the input transposes); fp32 derived
        identb = consts.tile([128, 128], BF)
        make_identity(nc, identb)
        ident = consts.tile([128, 128], DT)
        nc.vector.tensor_copy(out=ident, in_=identb)

        pos_emb_s = consts.tile([J, P], BF)
        nc.gpsimd.dma_start(out=pos_emb_s, in_=pos_emb.ap())
        posT = consts.tile([P, J], BF)
        nc.gpsimd.dma_start(out=posT, in_=pos_embT.ap())
        # hist staging rides the gpsimd queue (SP is full with cand+W1T)
        for hf in range(2):
            sl = slice(hf * 50, (hf + 1) * 50)
            src_h = _ap(hist_b_in.ap(), hf * H * D,
                        [[D, 50], [2 * H * D, 4], [1, D]])
            nc.gpsimd.dma_start(out=hist_all[sl, :, :], in_=src_h)

        # E[j, a] = pos_emb @ Wh2.T  (Wh2 = W1[:, 900:1000])
        E_s = consts.tile([J, A], BF)
        psE = ps.tile([J, A], DT, tag="ps")
        nc.tensor.matmul(psE, lhsT=posT[:, :], rhs=w1T_9[:, :],
                         start=True, stop=True)
        nc.vector.tensor_copy(out=E_s, in_=psE)

        b1row = consts.tile([1, A], DT)
        nc.sync.dma_start(out=b1row, in_=b1.ap())
        one11 = consts.tile([1, 1], DT)
        nc.vector.memset(one11, 1.0)

        # c0[a] = Wc2 @ pos_emb[1] + b1 as two per-partition bias columns
        c0col = consts.tile([100, 2], DT)
        for ac in range(2):
            asl = slice(ac * 100, (ac + 1) * 100)
            psc = ps.tile([100, 1], DT, tag="ps")
            nc.tensor.matmul(psc, lhsT=_r(w1T_4[:, asl]), rhs=_r(posT[:, 1:2]),
                             start=True, stop=False)
            nc.tensor.matmul(psc, lhsT=_r(b1row[:, asl]), rhs=_r(one11[:, :]),
                             start=False, stop=True)
            nc.scalar.copy(out=c0col[:, ac:ac + 1], in_=psc)

        w2col = consts.tile([100, 2], HDT)
        nc.gpsimd.dma_start(out=w2col,
                            in_=w2.ap().rearrange("(c p) -> p c", p=100))

        # mask bias (mask-1)*1e9 in [c, q, b, h] layout (broadcast over c, q)
        mb25 = consts.tile([NC2, 2, BC, H], DT)
        nc.gpsimd.dma_start(out=mb25, in_=_bc(_bc(mask_in.ap(), 0, 2), 0, NC2))
        nc.scalar.activation(out=mb25, in_=mb25, func=AF.Copy,
                             bias=-1e9, scale=1e9)

        # one-hot of positions, transposed: onehot[j, b*H+h] = (pos[b,h]==j)
        pos52 = consts.tile([J, BC * H], I32)
        nc.gpsimd.dma_start(out=pos52, in_=_bc(pos_in.ap(), 0, J))
        iot = consts.tile([J, BC * H], I32)
        nc.gpsimd.iota(iot, pattern=[[0, BC * H]], base=0, channel_multiplier=1)
        onehot_s = consts.tile([J, BC * H], BF)
        nc.vector.tensor_tensor(out=onehot_s, in0=iot, in1=pos52, op=ALU.is_equal)

        # history in natural layout for the final attention — late need, so
        # last on the gpsimd queue
        src_hf = _ap(hist_b_in.ap(), 0, [[D, H], [H * D, BC], [1, D]])
        nc.gpsimd.dma_start(out=histf_b[:, :, 0:D], in_=src_hf)

        # ---------------- transposes ----------------
        # per-group tiles (1 writer each) so gemm quarter q only waits for
        # its own group's transpose+evacuation
        candTg = [consts.tile([100, 4, 100], BF, name=f"candTg{g}")
                  for g in range(4)]
        histTg = [consts.tile([100, 4, 100], BF, name=f"histTg{g}")
                  for g in range(4)]
        for g in range(4 if "transp" not in SKIP else 0):
            ptc = ps.tile([100, 4, 100], BF, tag="ps")
            pth = ps.tile([100, 4, 100], BF, tag="ps")
            for k in range(4):
                nc.tensor.transpose(
                    ptc[:, k, :],
                    cand_all[:, g, k * 100:(k + 1) * 100],
                    identb[:100, :100])
                nc.tensor.transpose(
                    pth[:, k, :],
                    hist_all[:, g, k * 100:(k + 1) * 100],
                    identb[:100, :100])
            nc.scalar.copy(out=candTg[g], in_=ptc)
            nc.vector.tensor_copy(out=histTg[g], in_=pth)

        # candidate pos_emb[1] broadcast fill (fp32 -> bf16 cast on gpsimd)
        nc.gpsimd.dma_start(
            out=cand_out.ap()[:, :, D:F],
            in_=_bc(_bc(pos_emb.ap()[1:2, :], 0, N), 0, BC))

        # ------- GEMMs: hcT2[a, (b,n), dup2] (duplicated), hhT[a, (b,h)] ----
        hcT2 = consts.tile([100, 2, BC * N, 2], HDT)
        hhT = consts.tile([100, 2, BC * H], HDT)
        # batch-quartered gemm chunks so batch 0's hidden starts sooner
        NQ = 4
        for qtr in range(NQ if "gemm" not in SKIP else 0):
            csl = slice(qtr * BC * N // NQ, (qtr + 1) * BC * N // NQ)
            hsl = slice(qtr * BC * H // NQ, (qtr + 1) * BC * H // NQ)
            for ac in range(2):
                asl = slice(ac * 100, (ac + 1) * 100)
                pg = ps.tile([100, BC * N // NQ], DT, tag="ps")
                for k in range(4):
                    nc.tensor.matmul(pg, lhsT=_r(w1T_c[:, k, asl]),
                                     rhs=_r(candTg[qtr][:, k, :]),
                                     start=(k == 0), stop=(k == 3))
                # evacuate + add c0 bias, duplicating each element (dup2)
                nc.scalar.activation(out=hcT2[:, ac, csl, :],
                                     in_=_bc(pg[:, :], 2, 2),
                                     func=AF.Identity,
                                     bias=c0col[:, ac:ac + 1], scale=1.0)

                ph = ps.tile([100, BC * H // NQ], DT, tag="ps")
                for k in range(4):
                    nc.tensor.matmul(ph, lhsT=_r(w1T_h[:, k, asl]),
                                     rhs=_r(histTg[qtr][:, k, :]),
                                     start=(k == 0), stop=False)
                nc.tensor.matmul(ph, lhsT=_r(E_s[:, asl]),
                                 rhs=_r(onehot_s[:, hsl]),
                                 start=False, stop=True)
                # store NEGATED hh: hidden is rebuilt as max(hc, -hh); the
                # dropped +hh term is folded into the mask-bias tile below
                nc.scalar.activation(out=hhT[:, ac, hsl], in_=ph,
                                     func=AF.Copy, scale=-1.0)

        # S[(b,h)] = sum_a w2[a]*(-hh[a,(b,h)]); mbc = mb25 - S  (so the
        # softmax logits get +sum_a w2*hh added back, batched once)
        w2c25 = consts.tile([100, 2, NC2], HDT)
        for ac in range(2):
            nc.vector.tensor_copy(out=w2c25[:, ac, :],
                                  in_=_bc(w2col[:, ac:ac + 1], 1, NC2))
        psS = ps.tile([NC2, BC * H], DT, tag="ps")
        for ac in range(2):
            nc.tensor.matmul(psS, lhsT=w2c25[:, ac, :],
                             rhs=hhT[:, ac, :],
                             start=(ac == 0), stop=(ac == 1))
        mbc = consts.tile([NC2, 2, BC, H], DT)
        Ssb = consts.tile([NC2, BC * H], DT)
        nc.scalar.copy(out=Ssb, in_=psS)
        Ssb_b = _ap(Ssb[:, :], 0,
                    [list(Ssb[:, :].ap[0]), [0, 2], [H, BC], [1, H]])
        nc.gpsimd.tensor_tensor(out=mbc, in0=mb25, in1=Ssb_b,
                                op=ALU.subtract)

        # hist position columns gathered via one-hot matmuls (feeds the
        # final attention only — deprioritized behind the gemm evacs)
        for b in range(BC):
            ppg = ps.tile([H, P], DT, tag="ps")
            nc.tensor.matmul(ppg, lhsT=onehot_s[:, b * H:(b + 1) * H],
                             rhs=pos_emb_s[:, :], start=True, stop=True)
            nc.scalar.copy(out=histf_b[:, b, D:F], in_=ppg)

        # ---- per-batch fused pipeline: hidden max -> matvec -> softmax ->
        # ---- attention. No cross-batch barrier; engines pipeline batches.
        urs_all = consts.tile([NC2, 2, BC, F], BF)
        for b in range(BC):
            hids = []
            for ac in range(2):
                # hc broadcast over h via duplicated pairs: free = (n, hq, hr)
                v = hcT2[:, ac, b * N:(b + 1) * N, :]   # [[p],[2,50],[1,2]]
                hcb = _bc(v, 2, H // 2)                 # [100, 50, 25, 2]
                w = hhT[:, ac, b * H:(b + 1) * H]       # [[p],[1,50]]
                hhb = _ap(w, 0, [list(w.ap[0]), [0, N], [2, H // 2], [1, 2]])
                hid = hidp.tile([100, N, H // 2, 2], HDT, tag=f"hid{ac}")
                if "hidden" not in SKIP:
                    # DVE-only: gpsimd has no ucode for TT max, ACT can't TT
                    nc.vector.tensor_tensor(out=hid, in0=hcb, in1=hhb,
                                            op=ALU.max)
                hids.append(hid)
            amc = psm.tile([100, NC2], DT, tag="amc")
            for c in range(NC2 if "matvec" not in SKIP else 0):
                for ac in range(2):
                    hv = hids[ac]
                    pst = [list(x) for x in hv.ap][0]
                    lhs = _ap(hv, c * 100, [pst, [1, 100]])
                    nc.tensor.matmul(amc[:, c:c + 1], lhsT=_r(lhs),
                                     rhs=_r(w2col[:, ac:ac + 1]),
                                     start=(ac == 0), stop=(ac == 1))
            if "matvec" in SKIP:
                continue
            amcs = amcp.tile([100, NC2], DT, tag="amcs")
            nc.scalar.copy(out=amcs, in_=amc)
            amT = psm.tile([NC2, 2 * H], DT, tag="amT")
            nc.tensor.transpose(amT[:, :], _r(amcs[:, :]),
                                _r(ident[:100, :100]))
            # evacuate logits on ACT, then mask-add on gpsimd (SBUF only).
            # logits are O(10), far from exp overflow, so no max-subtract;
            # exp and its row-sum fuse into one ACT op per parity.
            amrb = smp.tile([NC2, 2, H], DT, tag="amr")
            amT_r = _ap(amT[:, :], 0, [list(amT[:, :].ap[0]), [H, 2], [1, H]])
            nc.scalar.copy(out=amrb, in_=amT_r)
            ammb = smp.tile([NC2, 2, H], DT, tag="amm")
            nc.gpsimd.tensor_add(out=ammb, in0=amrb, in1=mbc[:, :, b, :])
            exb = smp.tile([NC2, 2, H], BF, tag="ex")
            ssumb = smp.tile([NC2, 2], DT, tag="ssum")
            for q in range(2):
                nc.scalar.activation(out=exb[:, q, :], in_=ammb[:, q, :],
                                     func=AF.Exp,
                                     accum_out=ssumb[:, q:q + 1])
            rsb = smp.tile([NC2, 2], DT, tag="rs")
            nc.vector.reciprocal(rsb, ssumb)

            if "final" in SKIP:
                continue
            # both parities transposed into one PSUM tile (q1 at partition
            # 64 — PE tiles must start at 0/32/64/96), single evacuation
            for q in range(2):
                peT = ps.tile([H, NC2], BF, tag="ps")
                nc.tensor.transpose(peT[:, :], exb[:, q, :],
                                    identb[:NC2, :NC2])
                eT = eTp.tile([H, NC2], BF, tag="eT")
                nc.scalar.copy(out=eT, in_=peT)
                pur = ps.tile([NC2, F], DT, tag="ps")
                nc.tensor.matmul(pur, lhsT=eT[:, :],
                                 rhs=histf_b[:, b, :],
                                 start=True, stop=True)
                # normalize during PSUM evacuation (gpsimd cannot see PSUM);
                # DVE helps only on the last batches, after its max streak
                if q == 1 and b >= BC - 2:
                    nc.vector.tensor_scalar_mul(out=urs_all[:, q, b, :],
                                                in0=pur,
                                                scalar1=rsb[:, q:q + 1])
                else:
                    nc.scalar.activation(out=urs_all[:, q, b, :], in_=pur,
                                         func=AF.Copy, scale=rsb[:, q:q + 1])
            # stream this batch's slice out immediately (overlaps compute)
            dst_b = _ap(ur_out.ap(), b * N * F,
                        [[2 * F, NC2], [F, 2], [1, F]])
            nc.sync.dma_start(out=dst_b, in_=urs_all[:, :, b, :])



def build(debug=False, reps=1):
    nc = bacc.Bacc("TRN2", target_bir_lowering=False, debug=debug)
    hist_b_in = nc.dram_tensor("hist_b_in", [BC, H, D], BF,
                               kind="ExternalInput")
    cand_b_in = nc.dram_tensor("cand_b_in", [BC, N, D], BF,
                               kind="ExternalInput")
    mask_in = nc.dram_tensor("mask_in", [BC, H], DT, kind="ExternalInput")
    pos_in = nc.dram_tensor("pos_in", [BC, H], I32, kind="ExternalInput")
    pos_emb = nc.dram_tensor("pos_emb", [J, P], DT, kind="ExternalInput")
    w1t = nc.dram_tensor("w1t", [2 * F, A], BF, kind="ExternalInput")
    pos_embT = nc.dram_tensor("pos_embT", [P, J], DT, kind="ExternalInput")
    b1 = nc.dram_tensor("b1", [A], DT, kind="ExternalInput")
    w2 = nc.dram_tensor("w2", [A], DT, kind="ExternalInput")
    ur_out = nc.dram_tensor("ur_out", [BC, N, F], BF, kind="ExternalOutput")
    cand_out = nc.dram_tensor("cand_out", [BC, N, F], BF,
                              kind="ExternalOutput")

    with tile.TileContext(nc) as tc:
        for _ in range(reps):
            _body(nc, hist_b_in, cand_b_in, mask_in, pos_in,
                  pos_emb, w1t, pos_embT, b1, w2, ur_out, cand_out, tc)
    nc.compile()
    return nc


_NC = None


def _get_nc():
    global _NC
    if _NC is None:
        _NC = build(debug=False)
    return _NC


def make_in_maps(history_repr, candidate_repr, user_history_mask,
                 user_history_position, pos_emb, W1, b1, w2):
    import ml_dtypes
    hist = np.ascontiguousarray(np.asarray(history_repr, np.float32))
    cand = np.ascontiguousarray(np.asarray(candidate_repr, np.float32))
    hist_b = hist.astype(ml_dtypes.bfloat16)
    cand_b = cand.astype(ml_dtypes.bfloat16)
    mask = np.asarray(user_history_mask).astype(np.float32)
    pos = np.asarray(user_history_position).astype(np.int32)
    pe = np.ascontiguousarray(np.asarray(pos_emb, np.float32))
    import ml_dtypes
    w1t = np.ascontiguousarray(
        np.asarray(W1, np.float32).T.astype(ml_dtypes.bfloat16))
    peT = np.ascontiguousarray(pe.T)
    b1_ = np.ascontiguousarray(np.asarray(b1, np.float32))
    w2_ = np.ascontiguousarray(np.asarray(w2, np.float32))
    in_maps = []
    for c in range(NCORES):
        sl = slice(c * BC, (c + 1) * BC)
        in_maps.append({
            "hist_b_in": hist_b[sl], "cand_b_in": cand_b[sl],
            "mask_in": mask[sl], "pos_in": pos[sl],
            "pos_emb": pe, "w1t": w1t, "pos_embT": peT,
            "b1": b1_, "w2": w2_,
        })
    return in_maps


def kernel(history_repr, candidate_repr, user_history_mask,
           user_history_position, pos_emb, W1, b1, w2, b2=None, **_ignored):
    # b2 shifts every logit equally -> cancels in softmax; unused.
    nc = _get_nc()
    in_maps = make_in_maps(history_repr, candidate_repr, user_history_mask,
                           user_history_position, pos_emb, W1, b1, w2)
    res = bass_utils.run_bass_kernel_spmd(nc, in_maps, list(range(NCORES)))
    ur = np.concatenate(
        [np.asarray(res.results[c]["ur_out"]).astype(np.float32)
         for c in range(NCORES)], 0)
    cand = np.concatenate(
        [np.asarray(res.results[c]["cand_out"]).astype(np.float32)
         for c in range(NCORES)], 0)
    return ur, cand

